# revision 3
# baseline (speedup 1.0000x reference)
"""Trainium2 Bass kernel for nn_CrAKNVectorAttention2D.

Math: the reference ends with
    weight = softmax(..., axis=-2)            # normalize over j
    out    = einsum('ijk,ik->ik', weight, v)  # = v[i,k] * sum_j weight[i,j,k]
and sum_j softmax(x)[i,j,k] == 1 identically, so the entire pairwise
attention pipeline cancels and out == value == feat @ Wv.T + bv exactly
(up to fp32 rounding of the softmax sum).

The kernel therefore computes value = feat @ Wv.T + bv, data-parallel
over the N=2048 rows across 8 NeuronCores (256 rows/core). Layout is
transposed on host (feat.T), so each core runs a single K=128, M=128,
N=256 fp32 matmul (out_T = Wv @ feat_shard.T in PSUM) and evicts
PSUM -> SBUF with a per-partition bias add on ScalarE.
"""

import numpy as np

N, D = 2048, 128
NCORES = 8
RPC = N // NCORES  # rows per core

TRACE = False
LAST_RESULT = None

_cache = {}


def _install_profile_hook():
    """Restore NTFF profiling under axon: the image's antenv lacks
    axon_hooks, so boot() skipped hook registration. Inject the module
    and register the ctypes-based hook; stub out the artifact upload."""
    if _cache.get("hook_done"):
        return
    _cache["hook_done"] = True
    try:
        import sys
        import types

        import antenv

        if "antenv.axon_hooks" not in sys.modules:
            mod = types.ModuleType("antenv.axon_hooks")
            _hook = [None]
            mod.set_axon_ntff_profile_hook = lambda h: _hook.__setitem__(0, h)
            mod.get_axon_ntff_profile_hook = lambda: _hook[0]
            sys.modules["antenv.axon_hooks"] = mod
            antenv.axon_hooks = mod

        from antenv.axon_hooks import (
            get_axon_ntff_profile_hook,
            set_axon_ntff_profile_hook,
        )

        if get_axon_ntff_profile_hook() is None:
            from trn_agent_boot.trn_boot import _ntff_profile_via_ctypes

            set_axon_ntff_profile_hook(
                _ntff_profile_via_ctypes("/opt/axon/libaxon_pjrt.so")
            )

        import concourse.bass_utils as bu

        bu.upload_artifacts = lambda tmpdir: "local://" + str(tmpdir)
    except Exception as e:  # profiling is best-effort
        print(f"profile hook install failed: {type(e).__name__}: {e}")


def _get_nc():
    if "nc" in _cache:
        return _cache["nc"]
    import concourse.bacc as bacc
    import concourse.mybir as mybir
    import concourse.tile as tile

    nc = bacc.Bacc("TRN2", target_bir_lowering=False, debug=False)

    featT = nc.dram_tensor("featT", [D, RPC], mybir.dt.float32, kind="ExternalInput").ap()
    WvT = nc.dram_tensor("WvT", [D, D], mybir.dt.float32, kind="ExternalInput").ap()
    bv = nc.dram_tensor("bv", [D, 1], mybir.dt.float32, kind="ExternalInput").ap()
    outT = nc.dram_tensor("outT", [D, RPC], mybir.dt.float32, kind="ExternalOutput").ap()

    with tile.TileContext(nc) as tc:
        with (
            tc.tile_pool(name="sbuf", bufs=1) as pool,
            tc.tile_pool(name="psum", bufs=1, space="PSUM") as pp,
        ):
            ft = pool.tile([D, RPC], mybir.dt.float32)
            wt = pool.tile([D, D], mybir.dt.float32)
            bt = pool.tile([D, 1], mybir.dt.float32)
            nc.sync.dma_start(ft[:], featT[:])
            nc.sync.dma_start(wt[:], WvT[:])
            nc.sync.dma_start(bt[:], bv[:])

            ps = pp.tile([D, RPC], mybir.dt.float32)
            # out_T[j, n] = sum_k WvT[k, j] * featT[k, n] = (feat @ Wv.T).T
            nc.tensor.matmul(ps[:], wt[:], ft[:], start=True, stop=True)

            ot = pool.tile([D, RPC], mybir.dt.float32)
            nc.scalar.add(ot[:], ps[:], bt[:])  # + bv[j] per partition
            nc.sync.dma_start(outT[:], ot[:])

    nc.compile()
    _cache["nc"] = nc
    return nc


def kernel(**inputs) -> np.ndarray:
    global LAST_RESULT
    from concourse.bass_utils import run_bass_kernel_spmd

    feat = np.ascontiguousarray(np.asarray(inputs["feat"], dtype=np.float32))
    Wv = np.asarray(inputs["Wv"], dtype=np.float32)
    bv = np.asarray(inputs["bv"], dtype=np.float32)

    nc = _get_nc()

    featT = np.ascontiguousarray(feat.T)  # [D, N]
    WvT = np.ascontiguousarray(Wv.T)      # [D, D]; WvT[k, j] = Wv[j, k]
    bvc = np.ascontiguousarray(bv[:, None])  # [D, 1]

    in_maps = [
        {
            "featT": np.ascontiguousarray(featT[:, c * RPC : (c + 1) * RPC]),
            "WvT": WvT,
            "bv": bvc,
        }
        for c in range(NCORES)
    ]
    if TRACE:
        _install_profile_hook()
    res = run_bass_kernel_spmd(nc, in_maps, list(range(NCORES)), trace=TRACE)
    LAST_RESULT = res
    outT = np.concatenate([res.results[c]["outT"] for c in range(NCORES)], axis=1)
    return np.ascontiguousarray(outT.T)
